# revision 18
# baseline (speedup 1.0000x reference)
"""Trainium2 Bass kernel for nn_ExpertGather (MoE gather + per-expert GEMM).

Reference computation (B=8, T=8192, I=512, E=16, K=1024, J=512):
    gathered[b,e,k,:] = x[b, Ind[b,e,k], :]
    out[b,e,k,:]      = gathered[b,e,k,:] @ W[e]

Sharding: expert-parallel across 8 NeuronCores. Core c owns experts
[2c, 2c+1]; x is replicated, Ind/W/out are sharded on E.

Dataflow: the SWDGE dma_gather runs in transpose=True mode (requires
single_packet=False on hardware - single-packet transpose gathers crash the
exec unit), so gathered rows land in SBUF already transposed:
    gT[p, ic, k] = x[b, Ind[b,e,k], ic*128 + p]
i.e. feature-on-partitions - exactly the matmul lhsT layout. This removes
the PE identity-transposes (27us) and DVE PSUM->SBUF copies (50us) of the
previous dataflow, leaving PE with pure GEMM work:
  per (b, e_local) pair: 1 transposing gather (1 MiB), then per token tile
  tt (8): 4 accumulating matmuls (contraction I=512 in 128-chunks,
  lhsT = gT[:, ic, tt*128:...], rhs = W[e][ic] [128, 512]) -> fp32 PSUM
  [128tok, 512j]; ACT copy PSUM->SBUF fp16; one 1 MiB store per pair in
  p-major DRAM layout [128, KT, J] (host un-permutes to [K, J]).

Schedule details:
  - A ~430-matmul dummy warmup chain keeps PE continuously busy from t~1us
    through the pipeline-fill window, so every real matmul runs at the full
    2.4 GHz p-state (the ramp model otherwise costs ~11us).
  - idx DMA is issued before W so the first gather's descriptor generation
    overlaps the W load; pair 0's gather is split in half to start compute
    earlier; the last store is split to shorten the drain tail.

Engine budget per core (cost model): PE 109.2us (512 matmuls x 512 rows
@2.4GHz, zero inter-matmul gaps), DMA 95.2us (16 MiB gather + 16 MiB store
+ W), ACT ~73us (PSUM->SBUF), Pool 21.5us (SWDGE gen), DVE ~0.
TimelineSim: 122335 ns single-shot (baseline dataflow: 163102), steady-state
slope 109056 ns/iter. fp16 end-to-end accuracy: 4.7e-4 max-rel vs fp32
reference (gate 2e-2).
"""

import sys

import numpy as np

if "/opt/trn_rl_repo" not in sys.path:
    sys.path.insert(0, "/opt/trn_rl_repo")

B, T, I = 8, 8192, 512
E, K, J = 16, 1024, 512
NCORES = 8
E_LOCAL = E // NCORES  # 2 experts per core
PAIRS = B * E_LOCAL  # 16 (b, e_local) pairs per core
KT = K // 128  # 8 token tiles per pair
IC = I // 128  # 4 contraction chunks
IDX_W = K // 16  # 64 idxs per partition row (16-partition wrap)

N_WARM = 430  # PE p-state warmup matmuls covering the gather-fill window

_CACHE: dict = {}


def _build_nc(repeat=1):
    """Build the Bass module. `repeat` re-emits the whole computation that
    many times inside one NEFF (timing use only: slope between repeat counts
    cancels per-call dispatch overhead)."""
    import concourse.mybir as mybir
    import concourse.tile as tile
    from concourse import bacc

    f32 = mybir.dt.float32
    f16 = mybir.dt.float16
    i16 = mybir.dt.int16

    nc = bacc.Bacc("TRN2", target_bir_lowering=False, debug=False)
    x = nc.dram_tensor("x", [B, T, I], f16, kind="ExternalInput")
    w = nc.dram_tensor("w", [128, E_LOCAL, IC, J], f16, kind="ExternalInput")
    idx = nc.dram_tensor("idx", [128, PAIRS, IDX_W], i16, kind="ExternalInput")
    # p-major output layout: out[b, e, p, blk, j] = result[b, e, blk*128+p, j]
    out = nc.dram_tensor("out", [B, E_LOCAL, 128, KT, J], f16, kind="ExternalOutput")

    with tile.TileContext(nc) as tc:
        with (
            tc.tile_pool(name="const", bufs=1) as const_pool,
            tc.tile_pool(name="g", bufs=6) as g_pool,
            tc.tile_pool(name="osb", bufs=4) as o_pool,
            tc.tile_pool(name="ops", bufs=7, space="PSUM") as ops_pool,
            tc.tile_pool(name="warm", bufs=1, space="PSUM") as warm_pool,
        ):
            # idx first: the first gather depends on it; W loads can overlap
            # the first gather's descriptor generation.
            idx_sb = const_pool.tile([128, PAIRS, IDX_W], i16)
            nc.sync.dma_start(idx_sb[:], idx[:])
            w_sb = const_pool.tile([128, E_LOCAL, IC, J], f16)
            nc.sync.dma_start(w_sb[:], w[:])

            # PE p-state warmup: the cost of the ~3us ramp from cold is paid
            # by this dummy chain (tiny 32-row matmuls on a zeroed tile)
            # instead of by the first real matmuls; it also bridges the
            # pipeline-fill window so the real stream starts at full clock.
            dummy = const_pool.tile([128, 32], f16)
            nc.vector.memset(dummy[:], 0.0)
            warm_ps = warm_pool.tile([128, J], f32)
            for i in range(N_WARM):
                nc.tensor.matmul(
                    warm_ps[0:1, (i % 16) * 32 : (i % 16) * 32 + 32],
                    dummy[:, 0:1],
                    dummy[:, 0:32],
                    start=True,
                    stop=True,
                )

            for q in range(PAIRS * repeat):
                b, e = divmod(q % PAIRS, E_LOCAL)
                # transposing gather: gT[p, ic, k] = x[b, Ind[k], ic*128+p]
                if q == 0:
                    # split pair 0's gather (two half tiles) so compute
                    # starts half a gather earlier
                    half = K // 2
                    hw_ = IDX_W // 2
                    gt_halves = []
                    for h in range(2):
                        gh = g_pool.tile([128, IC, half], f16)
                        nc.gpsimd.dma_gather(
                            gh[:],
                            x[b],
                            idx_sb[:, 0, h * hw_ : (h + 1) * hw_],
                            half,
                            half,
                            I,
                            transpose=True,
                            single_packet=False,
                        )
                        gt_halves.append(gh)
                else:
                    gt = g_pool.tile([128, IC, K], f16)
                    nc.gpsimd.dma_gather(
                        gt[:],
                        x[b],
                        idx_sb[:, q % PAIRS, :],
                        K,
                        K,
                        I,
                        transpose=True,
                        single_packet=False,
                    )
                o_sb = o_pool.tile([128, KT, J], f16)
                for tt in range(KT):
                    if q == 0:
                        src = gt_halves[tt // 4]
                        kofs = (tt % 4) * 128
                    else:
                        src = gt
                        kofs = tt * 128
                    o_ps = ops_pool.tile([128, J], f32)
                    for ic in range(IC):
                        nc.tensor.matmul(
                            o_ps[:],
                            src[:, ic, kofs : kofs + 128],
                            w_sb[:, e, ic, :],
                            start=(ic == 0),
                            stop=(ic == IC - 1),
                        )
                    nc.scalar.copy(out=o_sb[:, tt, :], in_=o_ps[:])
                if q == PAIRS * repeat - 1:
                    # split the last store to shorten the drain tail
                    nc.sync.dma_start(out[b, e, :, 0 : KT // 2], o_sb[:, 0 : KT // 2])
                    nc.sync.dma_start(out[b, e, :, KT // 2 :], o_sb[:, KT // 2 :])
                else:
                    nc.sync.dma_start(out[b, e], o_sb[:])
    nc.compile()
    return nc


def _get_nc(repeat=1):
    key = ("nc", repeat)
    if key not in _CACHE:
        _CACHE[key] = _build_nc(repeat)
    return _CACHE[key]


def _make_in_maps(x, Ind, W):
    x = np.ascontiguousarray(np.asarray(x, dtype=np.float32).astype(np.float16))
    Ind = np.asarray(Ind)
    W = np.asarray(W, dtype=np.float32)
    in_maps = []
    for c in range(NCORES):
        wl = W[c * E_LOCAL : (c + 1) * E_LOCAL]  # [E_LOCAL, I, J]
        # w_host[p, e, ic, j] = wl[e, ic*128 + p, j]
        w_host = np.ascontiguousarray(
            wl.reshape(E_LOCAL, IC, 128, J).transpose(2, 0, 1, 3)
        ).astype(np.float16)
        idxs = np.empty((128, PAIRS, IDX_W), np.int16)
        for b in range(B):
            for e in range(E_LOCAL):
                q = b * E_LOCAL + e
                # unwrapped[j] = idxs[j % 16, j // 16]  (16-partition wrap)
                wrapped = Ind[b, c * E_LOCAL + e].astype(np.int16).reshape(IDX_W, 16).T
                idxs[:, q, :] = np.tile(wrapped, (8, 1))
        in_maps.append({"x": x, "w": w_host, "idx": idxs})
    return in_maps


def run(x, Ind, W, trace=False):
    """Run the kernel; returns (out, BassKernelResults)."""
    import os

    from concourse.bass_utils import run_bass_kernel_spmd

    nc = _get_nc()
    in_maps = _make_in_maps(x, Ind, W)
    try:
        res = run_bass_kernel_spmd(
            nc, in_maps, core_ids=list(range(NCORES)), trace=trace
        )
    except ModuleNotFoundError:
        # axon NTFF profiling hook absent (no antenv.axon_hooks) — retry
        # with tracing force-disabled.
        os.environ["BASS_NEVER_TRACE"] = "1"
        res = run_bass_kernel_spmd(
            nc, in_maps, core_ids=list(range(NCORES)), trace=False
        )
    outs = [r["out"] for r in res.results]  # each [B, E_LOCAL, 128, KT, J]
    # un-permute p-major layout -> [B, E_LOCAL, K, J], concat experts
    outs = [o.transpose(0, 1, 3, 2, 4).reshape(B, E_LOCAL, K, J) for o in outs]
    full = np.concatenate(outs, axis=1)  # experts in core order -> [B, E, K, J]
    return np.ascontiguousarray(full.astype(np.float32)), res


def kernel(x, Ind, W):
    out, _ = run(x, Ind, W, trace=False)
    return out
